# revision 1
# baseline (speedup 1.0000x reference)
"""Space-to-depth (8x8 chessboard) kernel for Trainium2.

Full input  : (32, 256, 256, 32) f32
Full output : (32, 8, 8, 32768) f32
out[b, i, j] = inputs[b, i*32:(i+1)*32, j*32:(j+1)*32, :].reshape(-1)

Sharding: batch dim (32) split across 8 NeuronCores (pure data parallel,
no communication) -> 4 examples per core.

Per core the op is pure HBM->HBM data movement, done entirely with DMA
access patterns (no compute engines). Key layout fact: within one
(example b, 32-row band i), iterating (r, j, elem) makes the source AP
contiguous and the destination a 3D AP, so a single DMA moves a
half-band (16 rows = 512 KiB) in 4 KiB contiguous chunks:

  src [[8192, nr], [1024, 8], [1, 1024]]   (contiguous 32 KiB per row r)
  dst [[1024, nr], [32768, 8], [1, 1024]]  (4 KiB chunks, 32 KiB stride)

Performance notes (measured on trn2 via NTFF traces):
- SDMA engine assignment is (outer AP dim index) mod 16, so outer count
  >= 16 engages all 16 SDMA engines (outer 8 uses only engines 0-7).
- Keep HWDGE DMAs at <= 128 descriptors (outer <= 16): outer 31/32 DMAs
  hit a slow descriptor-generation fallback that blocks the issuing
  sequencer 10-100 us per instruction and starves the engines (6x slower).
- 4 KiB descriptors outperform 32 KiB ones (~320 vs ~213 GB/s payload).
- Issuing from both HWDGE queues (sync=SP + scalar=ACT) beats one queue.
- SDMA engine 15 is intermittently ~1.25x slower (known trn2 quirk), so
  the job list is skewed: 16 of the 64 half-band DMAs carry 15 rows
  instead of 16 (their unit 15 would land on engine 15), and the 16
  skipped rows are covered by two batched "orphan" DMAs whose outer
  count 8 lands on engines 0-7. Engine 15 ends up with ~73% of the
  average load and no longer straggles.

Steady state ~320 GB/s payload (~640 GB/s HBM read+write traffic per
core) with all 8 cores running - measured equal to a plain contiguous
HBM->HBM copy, i.e. the permutation itself is free and the kernel runs
at the achievable DMA/HBM roofline. HW exec ~115-118 us per core.
"""

import numpy as np

_B_PER_CORE = 4
_N_CORES = 8
_IN_SHAPE = (_B_PER_CORE, 256, 256, 32)
_OUT_SHAPE = (_B_PER_CORE, 8, 8, 32768)
_EX = 256 * 256 * 32      # elements per example  (2097152)
_BAND = 32 * 256 * 32     # elements per (example, row-band)  (262144)

_CACHE = {}


def build_nc():
    import concourse.bass as bass
    import concourse.mybir as mybir

    nc = bass.Bass(target_bir_lowering=False)
    x = nc.dram_tensor("x", list(_IN_SHAPE), mybir.dt.float32, kind="ExternalInput")
    y = nc.dram_tensor("y", list(_OUT_SHAPE), mybir.dt.float32, kind="ExternalOutput")

    # Job list: half-band DMAs, then the two orphan batches last — each
    # queue ends on a small 256 KiB DMA, so the final completion fence
    # covers less data (finer drain quantum). For b in {0,1} the second
    # half-band is shortened to 15 rows (engine-15 skew).
    jobs = [
        (b, i, h * 16, 15 if (h == 1 and b < 2) else 16)
        for b in range(_B_PER_CORE)
        for i in range(8)
        for h in range(2)
    ] + [("orph", 0), ("orph", 1)]

    def issue(engine, my_jobs, sem):
        n = 0
        for job in my_jobs:
            if job[0] == "orph":
                # rows r=31 of all 8 bands of example b; one 32 KiB unit
                # per band -> outer count 8 -> SDMA engines 0-7
                _, b = job
                src = bass.AP(
                    x, b * _EX + 31 * 8192, [[262144, 8], [1024, 8], [1, 1024]]
                )
                dst = bass.AP(
                    y, b * _EX + 31 * 1024, [[262144, 8], [32768, 8], [1, 1024]]
                )
            else:
                b, i, r0, nr = job
                off = b * _EX + i * _BAND
                src = bass.AP(
                    x, off + r0 * 8192, [[8192, nr], [1024, 8], [1, 1024]]
                )
                dst = bass.AP(
                    y, off + r0 * 1024, [[1024, nr], [32768, 8], [1, 1024]]
                )
            engine.dma_start(out=dst, in_=src).then_inc(sem, 16)
            n += 16
        if n:
            engine.wait_ge(sem, n)

    with (
        nc.semaphore("sp_sem") as sp_sem,
        nc.semaphore("act_sem") as act_sem,
        nc.Block(no_gpsimd_drain=True) as block,
    ):

        @block.sync
        def _(sync):
            issue(sync, jobs[0::2], sp_sem)

        @block.scalar
        def _(scalar):
            issue(scalar, jobs[1::2], act_sem)

    return nc


def _get_nc():
    if "nc" not in _CACHE:
        _CACHE["nc"] = build_nc()
    return _CACHE["nc"]


def kernel(inputs: np.ndarray) -> np.ndarray:
    from concourse.bass_utils import run_bass_kernel_spmd

    inputs = np.ascontiguousarray(np.asarray(inputs, dtype=np.float32))
    assert inputs.shape == (_B_PER_CORE * _N_CORES,) + _IN_SHAPE[1:]

    nc = _get_nc()
    in_maps = [
        {"x": np.ascontiguousarray(inputs[c * _B_PER_CORE : (c + 1) * _B_PER_CORE])}
        for c in range(_N_CORES)
    ]
    res = run_bass_kernel_spmd(nc, in_maps, core_ids=list(range(_N_CORES)))
    return np.concatenate([r["y"] for r in res.results], axis=0)

